# revision 47
# baseline (speedup 1.0000x reference)
"""FP4Net (bnb-FP4 quantize-dequantize 4-layer MLP) Trainium2 kernel.

Strategy (8 NeuronCores):
  - Data-parallel over batch for the matmuls: each core handles 1024 of 8192 rows.
  - FP4 quant-dequant of the weights is sharded 8x across cores (by output-row
    blocks, keeping the 64-elem FP4 blocks intact), computed exactly with fp32
    bit tricks on the vector engine, stored transposed (W.T layout) in fp16,
    then AllGathered so every core has all dequantized weights.
  - Each weight's AllGather is split into single-128-row-block chunks (w1/w2/
    w3: 4 each, w4: 1); chunk ci gathers j-tiles {c*nrt+r for cores c, r in
    chunk}, and each layer's j-loop walks chunks in order, so matmuls start as
    soon as the first chunk lands. Each AllGather is emitted on the gpsimd
    queue immediately after its chunk's stores (earliest possible arm; the
    mesh start is bounded by ~45us of cross-core launch skew, not by us).
    Short chunks keep each collective's HBM-saturating transfer window small
    so the PE's weight-strip DMAs recover between windows.
  - Queue discipline: Vector = dequant math only; Scalar(ACT) = dequant input
    loads + x/bias staging + epilogues; GpSimd = stores + AllGathers;
    SP(sync) = weight-strip loads + output stores (pure PE feed).
  - Dequant-side work is emitted into the layer loops by estimated ready time,
    so no queue head-of-line blocks on a dependency that isn't about to be
    satisfied.
  - Matmuls measure 262.6ns per [128,128]x[128,512] on this platform: a
    SW/GPIO HAM throttle caps the PE at 13/16 duty (1.95GHz) under sustained
    8-core load, and any PE idle >3.4us re-throttles to 4/8 with a slow
    recovery -- so the schedule optimizes for an unbroken matmul stream:
    ~270 garbage warm-up matmuls bridge the collective-bound startup (PE
    would otherwise idle ~60us), deep strip double-buffering (wt 5, wt1 10)
    rides out DMA starvation during AllGather windows, and redundant
    per-matmul LDWEIGHTS are deduped post-scheduling (_dedup_ldweights).

Rounding trick: with g = 3*w/scale, the bnb FP4 codebook {0, 1/192, 1/6, 1/4,
1/3, 1/2, 2/3, 1} maps to {0, 1/64, 1/2, 3/4, 1, 3/2, 2, 3}: round-to-nearest
over that set == round g to 1 stored mantissa bit (round-half-up via exact
small-significand integer adds), clamped below at 1/2, plus a two-threshold
step for the {0, 1/64} region. Verified bit-exact vs the jax reference modulo
~1-ulp boundary fuzz (~1 flipped element per 16M weights on the actual data).
"""
import os
import sys
import numpy as np

for _p in ("/opt/trn_rl_repo", "/root/.axon_site/_ro/trn_rl_repo"):
    if _p not in sys.path:
        sys.path.append(_p)

N_CORES = 8
B, IN, H, OUT = 8192, 1024, 4096, 1024
BS = B // N_CORES          # batch shard per core
HS = H // N_CORES          # hidden-row shard per core (w1/w2/w3)
OS = OUT // N_CORES        # out-row shard per core (w4)

# FP4 codebook-derived threshold constants (g-space = 3*norm), f64 precision
_FP4_POS = np.array([0.0, 0.0052083333, 0.6666667, 1.0, 0.3333333, 0.5,
                     0.1666667, 0.25], dtype=np.float32)
_CS = np.sort(_FP4_POS).astype(np.float64)
_TL = float(np.float32(3.0 * (_CS[0] + _CS[1]) / 2.0))
_TH = float(np.float32(3.0 * (_CS[1] + _CS[2]) / 2.0))
LO_BITS = int(np.float32(1.0 / 64).view(np.uint32))   # 0x3C800000
BIG_BITS = 0x40400000                                  # bits of 3.0


def _i32(x):
    return int(np.uint32(x).view(np.int32))


_CACHED = {}

# weight dims per layer: (rows of W == dout, k == contraction)
WDIMS = {1: (H, IN), 2: (H, H), 3: (H, H), 4: (OUT, H)}
NRT = {l: (d // N_CORES) // 128 for l, (d, _k) in WDIMS.items()}  # r-blocks
CHUNKS = {1: [[0], [1], [2], [3]], 2: [[0], [1], [2], [3]],
          3: [[0], [1], [2], [3]], 4: [[0]]}      # r-blocks per AllGather chunk
# k-segments per (layer, chunk): AllGather k-splitting (unused -- splitting
# w2c0 made its tail land LATER due to CC serialization + per-AG setup cost)
SEGS = {}
FDQ = 512          # dequant tile free-size (fp32 elems per partition)
NBQ = FDQ // 64    # fp4 blocks per tile

# --- static timing model (us) used only to order emission ---
DVE_TILE_US = 7.5          # dequant DVE time per [128, FDQ] tile
DVE_T0_US = 5.0            # engine init before first dequant op
MM_US = 0.2626             # per N=512 matmul at HAM 13/16 duty
L1_START_US = 46.0         # estimated first-epilogue time of layer 1
EV_MARGIN_US = 3.0


def _build_nc(taps=False):
    import concourse.bass as bass
    import concourse.mybir as mybir
    import concourse.tile as tile
    from concourse import bacc

    dt = mybir.dt
    Alu = mybir.AluOpType
    Act = mybir.ActivationFunctionType

    nc = bacc.Bacc("TRN2", target_bir_lowering=False, debug=False,
                   num_devices=N_CORES)

    # ---- I/O ----
    xs = nc.dram_tensor("xst", [IN, BS], dt.float16, kind="ExternalInput")
    w_in = {
        1: nc.dram_tensor("w1s", [HS, IN], dt.float32, kind="ExternalInput"),
        2: nc.dram_tensor("w2s", [HS, H], dt.float32, kind="ExternalInput"),
        3: nc.dram_tensor("w3s", [HS, H], dt.float32, kind="ExternalInput"),
        4: nc.dram_tensor("w4s", [OS, H], dt.float32, kind="ExternalInput"),
    }
    b_in = {
        1: nc.dram_tensor("b1", [128, H // 128], dt.float32, kind="ExternalInput"),
        2: nc.dram_tensor("b2", [128, H // 128], dt.float32, kind="ExternalInput"),
        3: nc.dram_tensor("b3", [128, H // 128], dt.float32, kind="ExternalInput"),
        4: nc.dram_tensor("b4", [128, OUT // 128], dt.float32, kind="ExternalInput"),
    }
    y_out = nc.dram_tensor("y", [OUT, BS], dt.float32, kind="ExternalOutput")


    # ---- internal DRAM: per-chunk-segment shard + AllGather outputs ----
    dq_shard = {}   # (l, ci, s) -> local tensor [len(chunk), kseg, 128]
    dq_chunk = {}   # (l, ci, s) -> gathered [N_CORES*len(chunk), kseg, 128]
    for l, (dout, k) in WDIMS.items():
        for ci, chunk in enumerate(CHUNKS[l]):
            ns = SEGS.get((l, ci), 1)
            for s in range(ns):
                dq_shard[(l, ci, s)] = nc.dram_tensor(
                    f"dqs{l}{ci}{s}", [len(chunk), k // ns, 128], dt.float16)
                dq_chunk[(l, ci, s)] = nc.dram_tensor(
                    f"dqc{l}{ci}{s}", [N_CORES * len(chunk), k // ns, 128],
                    dt.float16, addr_space="Shared")

    with tile.TileContext(nc) as tc:
        with (
            tc.tile_pool(name="const", bufs=1) as cpool,
            tc.tile_pool(name="bias", bufs=1) as bpool,
            tc.tile_pool(name="a0", bufs=1) as a0pool,
            tc.tile_pool(name="acts", bufs=2) as apool,
            tc.tile_pool(name="dqin", bufs=3) as dqin_pool,
            tc.tile_pool(name="dqtmp", bufs=1) as dqtmp_pool,
            tc.tile_pool(name="dqout", bufs=3) as dqout_pool,
            tc.tile_pool(name="stp", bufs=3) as stp_pool,
            tc.tile_pool(name="wt", bufs=5) as wpool,
            tc.tile_pool(name="wt1", bufs=10) as wpool1,
            tc.tile_pool(name="psum", bufs=8, space="PSUM") as pspool,
        ):
            # int32 constants for scalar_tensor_tensor scalars
            c_half = cpool.tile([128, 1], dt.int32)
            nc.vector.memset(c_half[:], _i32(0x3F000000))
            c_sign = cpool.tile([128, 1], dt.int32)
            nc.vector.memset(c_sign[:], _i32(0x80000000))
            wz = cpool.tile([128, 128], dt.float16)
            nc.vector.memset(wz[:], 0.0)

            # ---- dequant pipeline pieces ----
            # tile order: (l, r, cix) over all weights, r-block-major
            all_tiles = []
            for l in WDIMS:
                for r in range(NRT[l]):
                    for cix in range(WDIMS[l][1] // FDQ):
                        all_tiles.append((l, r, cix))
            t_index = {t: i for i, t in enumerate(all_tiles)}

            def dve_done(i):  # estimated DVE completion time of tile i (us)
                return DVE_T0_US + (i + 1) * DVE_TILE_US

            def emit_dq_load(l, r, cix):
                """Scalar queue: stage [128 rows, FDQ] fp32 of weight l."""
                w = dqin_pool.tile([128, NBQ, 64], dt.float32, tag="dqw")
                nc.scalar.dma_start(
                    w[:],
                    w_in[l][r * 128:(r + 1) * 128, cix * FDQ:(cix + 1) * FDQ]
                    .rearrange("p (b i) -> p b i", i=64))
                return w

            def emit_dq_vec_pair(tiles, staged):
                """Vector queue: dequant math + 32x32 stream-transpose for a
                pair of tiles, op-interleaved so the per-op semaphore latency
                of one tile's chain hides behind the other tile's op."""
                ss = []
                for par, t in enumerate(tiles):
                    w = staged.pop(t)

                    def tmp(nm, shape, dty, par=par):
                        return dqtmp_pool.tile(shape, dty, name=f"{nm}{par}",
                                               tag=f"{nm}{par}")
                    ss.append({"w": w, "tmp": tmp})

                def each(fn):
                    for s in ss:
                        fn(s)

                def st_scale(s):
                    s["scale"] = s["tmp"]("scale", [128, NBQ, 1], dt.float32)
                    nc.vector.tensor_reduce(
                        s["scale"][:], s["w"][:], axis=mybir.AxisListType.X,
                        op=Alu.max, apply_absolute_value=True)
                each(st_scale)

                def st_recip(s):
                    s["recip"] = s["tmp"]("recip", [128, NBQ, 1], dt.float32)
                    nc.vector.reciprocal(s["recip"][:], s["scale"][:])
                each(st_recip)

                def st_s3(s):
                    s["s3"] = s["tmp"]("s3", [128, NBQ, 1], dt.float32)
                    nc.vector.tensor_scalar_mul(s["s3"][:], s["scale"][:],
                                                1.0 / 3.0)
                each(st_s3)

                def st_g(s):
                    s["g"] = s["tmp"]("g", [128, NBQ, 64], dt.float32)
                    nc.vector.scalar_tensor_tensor(
                        s["g"][:], s["w"][:], 3.0,
                        s["recip"][:].broadcast_to((128, NBQ, 64)),
                        op0=Alu.mult, op1=Alu.mult)
                each(st_g)

                # NOTE: DVE ops must never write in-place onto their own input
                # (dual-port perf modes race), and int adds must keep few
                # significant bits (the int ALU path is fp32-internal).
                def st_ta(s):
                    s["ta"] = s["tmp"]("ta", [128, NBQ, 64], dt.int32)
                    nc.vector.tensor_scalar(
                        s["ta"][:], s["g"][:].bitcast(dt.int32),
                        _i32(0x7FFFFFFF), None,
                        op0=Alu.bitwise_and)  # m0 = |g| bits
                each(st_ta)

                def st_tb(s):
                    s["tb"] = s["tmp"]("tb", [128, NBQ, 64], dt.int32)
                    nc.vector.tensor_scalar(s["tb"][:], s["ta"][:],
                                            _i32(0xFFC00000), None,
                                            op0=Alu.bitwise_and)  # trunc
                each(st_tb)

                def st_tc(s):
                    s["tc"] = s["tmp"]("tc", [128, NBQ, 64], dt.int32)
                    nc.vector.tensor_scalar(
                        s["tc"][:], s["ta"][:], _i32(0x00200000), _i32(1),
                        op0=Alu.bitwise_and,
                        op1=Alu.logical_shift_left)  # half-bit<<1
                each(st_tc)

                def st_te(s):
                    s["te"] = s["tmp"]("te", [128, NBQ, 64], dt.int32)
                    nc.vector.tensor_tensor(
                        s["te"][:], s["tb"][:], s["tc"][:],
                        op=Alu.add)  # r2a (exact: 10+1 sig bits)
                each(st_te)

                # M1L = (|g|>TL)*LO_BITS, M2B = (|g|>TH)*BIG_BITS -- the float
                # products are exact (consts have <=5 significant bits)
                def st_m1(s):
                    af = s["ta"][:].bitcast(dt.float32)  # |g| as float
                    nc.vector.tensor_scalar(s["tb"][:], af, _TL,
                                            float(LO_BITS),
                                            op0=Alu.is_gt, op1=Alu.mult)
                each(st_m1)

                def st_m2(s):
                    # ta (|g| bits) has no reader after this: m2 -> tc and the
                    # select result can overwrite ta, freeing the former td
                    # buffer entirely (SBUF pressure).
                    af = s["ta"][:].bitcast(dt.float32)
                    nc.vector.tensor_scalar(s["tc"][:], af, _TH,
                                            float(BIG_BITS),
                                            op0=Alu.is_gt, op1=Alu.mult)
                each(st_m2)

                def st_sel(s):
                    nc.vector.tensor_tensor(s["ta"][:], s["tb"][:], s["tc"][:],
                                            op=Alu.add)  # sel (disjoint bits)
                each(st_sel)

                def st_mag(s):
                    nc.vector.scalar_tensor_tensor(
                        s["tb"][:], s["te"][:], c_half[:], s["ta"][:],
                        op0=Alu.max, op1=Alu.min)  # mag
                each(st_mag)

                def st_sign(s):
                    nc.vector.scalar_tensor_tensor(
                        s["tc"][:], s["g"][:].bitcast(dt.int32), c_sign[:],
                        s["tb"][:],
                        op0=Alu.bitwise_and, op1=Alu.bitwise_or)  # signed
                each(st_sign)

                def st_dq(s):
                    s["dq"] = dqout_pool.tile([128, NBQ, 64], dt.float16,
                                              name="dq", tag="dq")
                    nc.vector.tensor_tensor(
                        s["dq"][:], s["tc"][:].bitcast(dt.float32),
                        s["s3"][:].broadcast_to((128, NBQ, 64)), op=Alu.mult)
                each(st_dq)

                # 32x32 block transpose on the DVE reshape front-end:
                # st[32a+u, 32b+v] = dq[32a+v, 32b+u]
                def st_tp(s):
                    s["st"] = stp_pool.tile([128, FDQ], dt.float16,
                                            name="st", tag="st")
                    nc.vector.transpose(
                        s["st"][:], s["dq"][:].rearrange("p b i -> p (b i)"))
                each(st_tp)
                return [s["st"] for s in ss]

            ntk = {l: WDIMS[l][1] // FDQ for l in WDIMS}

            def chunk_of(l, r):
                return next(c for c, ch in enumerate(CHUNKS[l]) if r in ch)

            def seg_of(l, ci, cix):
                ns = SEGS.get((l, ci), 1)
                return cix // (ntk[l] // ns)

            def emit_dq_store(l, ci, r, cix, st, split=False):
                """GpSimd: 4 block-grid-swapped stores complete the transpose.
                dst[k=cix*FDQ+32b+u, h=32a+v] = st[32a+u, 32b+v]. For the
                first tiles (split=True) half the stores ride the still-idle
                sync queue, so the first AllGather's data is ready sooner."""
                ri = r - CHUNKS[l][ci][0]
                s = seg_of(l, ci, cix)
                cis = cix - s * (ntk[l] // SEGS.get((l, ci), 1))
                for a in range(4):
                    q = nc.sync if (split and a % 2) else nc.gpsimd
                    q.dma_start(
                        dq_shard[(l, ci, s)][ri, cis * FDQ:(cis + 1) * FDQ,
                                             32 * a:32 * (a + 1)]
                        .rearrange("(f p) h -> p f h", p=32),
                        st[32 * a:32 * (a + 1), :]
                        .rearrange("p (f i) -> p f i", i=32))

            def emit_ag(l, ci, s):
                nc.gpsimd.collective_compute(
                    "AllGather", Alu.bypass,
                    replica_groups=[list(range(N_CORES))],
                    ins=[dq_shard[(l, ci, s)][:]],
                    outs=[dq_chunk[(l, ci, s)][:]],
                )

            # ---- event list: (ready_us, seq, kind, payload), ready-ordered.
            # AllGathers are NOT events: each is emitted inline right after
            # the stores of its chunk-segment's last tile, so it sits in the
            # gpsimd queue (and thus the CC engine) at the earliest moment
            # its data can be complete -- no head-of-line behind later
            # tiles' stores.
            events = []
            seq = 0
            N_HOIST = 3   # dq loads emitted before x/bias staging
            for t in all_tiles:
                i = t_index[t]
                rdy_load = 0.0 if i < 4 else dve_done(i - 4)
                if i >= N_HOIST:
                    events.append((rdy_load, seq, "load", t)); seq += 1
                if i % 2 == 1:
                    events.append((rdy_load, seq, "vec",
                                   (all_tiles[i - 1], t))); seq += 1
            events.sort(key=lambda e: (e[0], e[1]))
            events = events[::-1]  # pop from end

            staged = {}   # tile -> dqin handle

            def flush_events(now_us):
                while events and events[-1][0] <= now_us:
                    _, _, kind, payload = events.pop()
                    if kind == "load":
                        staged[payload] = emit_dq_load(*payload)
                    else:
                        sts = emit_dq_vec_pair(payload, staged)
                        for t, st in zip(payload, sts):
                            l, r, cix = t
                            ci = chunk_of(l, r)
                            emit_dq_store(l, ci, r, cix, st,
                                          split=t_index[t] < 4)
                            ns = SEGS.get((l, ci), 1)
                            s = seg_of(l, ci, cix)
                            if (r == CHUNKS[l][ci][-1]
                                    and cix == (s + 1) * (ntk[l] // ns) - 1):
                                emit_ag(l, ci, s)

            # loads for the first tiles go out before x/bias staging so the
            # dequant chain (the critical path to the first AllGather) starts
            # as early as possible
            for t in all_tiles[:N_HOIST]:
                staged[t] = emit_dq_load(*t)

            # ---- biases -> SBUF [128, ntiles]; x -> SBUF a0 (Scalar queue) ----
            b_sb = {}
            for l, (dout, _k) in WDIMS.items():
                nj = dout // 128
                bt = bpool.tile([128, nj], dt.float32, tag=f"bias{l}")
                nc.scalar.dma_start(bt[:], b_in[l][:])
                b_sb[l] = bt
            a_cur = a0pool.tile([128, IN // 128, BS], dt.float16)
            nc.scalar.dma_start(
                a_cur[:], xs[:].rearrange("(i p) b -> p i b", p=128))

            # everything ready before the first epilogue goes out up front
            flush_events(L1_START_US)

            # ---- HAM warm-up: garbage matmuls through the collective-bound
            # startup window (PE would idle ~80us waiting for the first
            # AllGather; idle >3.4us re-throttles the PE clock to 4/8, taxing
            # the first real j-blocks). Zero weights, x as moving operand,
            # results discarded. The ldweights dedup collapses these to one
            # weight load.
            for _wu in range(270):
                ps_w = pspool.tile([128, 512], dt.float32, tag="ps")
                nc.tensor.matmul(ps_w[:], wz[:], a_cur[:, 0, 0:512],
                                 start=True, stop=True)

            # ---- matmul layers, chunk-ordered j loops ----
            t_us = L1_START_US
            for l, (dout, K) in WDIMS.items():
                nj = dout // 128
                nk = K // 128
                nrt = NRT[l]
                half = nk // 2
                out_dt = dt.float32 if l == 4 else dt.float16
                a_next = apool.tile([128, nj, BS], out_dt, tag="acts")
                j_period = nk * (BS // 512) * MM_US
                for ci, chunk in enumerate(CHUNKS[l]):
                    for c in range(N_CORES):
                        for ri, r in enumerate(chunk):
                            j = c * nrt + r
                            row = c * len(chunk) + ri
                            src = dq_chunk[(l, ci, 0)][row]
                            wts = []
                            for si, i0 in enumerate((0, half)):
                                pool_l = wpool1 if l == 1 else wpool
                                wt_h = pool_l.tile([128, half, 128],
                                                   dt.float16,
                                                   tag="wt1" if l == 1
                                                   else "wt")
                                # second strip rides the (mostly idle) scalar
                                # queue: two independent queue positions in
                                # the DMA arbiter halve strip starvation
                                # during AllGather HBM-saturation windows
                                q = nc.sync if (l == 1 or si == 0) \
                                    else nc.scalar
                                q.dma_start(
                                    wt_h[:],
                                    src[i0 * 128:(i0 + half) * 128, :]
                                    .rearrange("(i p) h -> p i h", p=128))
                                wts.append(wt_h)
                            ps = []
                            for _n in range(BS // 512):
                                ps_t = pspool.tile([128, 512], dt.float32,
                                                   tag="ps")
                                ps.append(ps_t)
                            for i in range(nk):
                                for n in range(BS // 512):
                                    nc.tensor.matmul(
                                        ps[n][:], wts[i // half][:, i % half, :],
                                        a_cur[:, i, n * 512:(n + 1) * 512],
                                        start=(i == 0), stop=(i == nk - 1))
                            t_us += j_period
                            flush_events(t_us)
                            act_fn = Act.Sigmoid if l == 4 else Act.Relu
                            for n in range(BS // 512):
                                nc.scalar.activation(
                                    a_next[:, j, n * 512:(n + 1) * 512],
                                    ps[n][:], act_fn,
                                    bias=b_sb[l][:, j:j + 1], scale=1.0)
                                if l == 4:
                                    nc.sync.dma_start(
                                        y_out[j * 128:(j + 1) * 128,
                                              n * 512:(n + 1) * 512],
                                        a_next[:, j, n * 512:(n + 1) * 512])
                a_cur = a_next
            flush_events(1e9)

    _dedup_ldweights(nc, mybir)
    nc.compile()
    return nc


def _dedup_ldweights(nc, mybir):
    """Remove back-to-back duplicate InstLdweights (same weights AP).

    The tile scheduler pairs every InstMatmult with its own InstLdweights,
    but the PE's background weight buffer only frees when the next matmul
    issues (buffer flip), serializing flip->ldweights(110ns)->handshake
    (62ns) = 263ns per matmul. Our two batch-half matmuls share a weight
    tile: dropping the second (redundant) load makes the second matmul
    flip-free and hides the next load under 2x213ns of array work, so the
    stream becomes array-bound. Waits on a dropped load migrate to the
    following matmul; program-order weight semantics are preserved.
    """
    fn = nc.m.functions[0]
    removed = 0
    for blk in fn.blocks:
        insts = list(blk.instructions)
        last_sig = None
        pending_waits = None
        to_delete = []
        for i, inst in enumerate(insts):
            nm = type(inst).__name__
            if nm == "InstLdweights":
                sig = str(inst.ins[0])
                if sig == last_sig:
                    si = inst.sync_info
                    if si is not None and len(si.on_update) > 0:
                        last_sig = sig  # has updates: keep it, don't merge
                        continue
                    if si is not None and len(si.on_wait) > 0:
                        pending_waits = list(si.on_wait)
                    to_delete.append(i)
                else:
                    last_sig = sig
            elif nm == "InstMatmult":
                if pending_waits:
                    msi = inst.sync_info
                    if msi is None:
                        inst.sync_info = mybir.SyncInfo(
                            on_wait=pending_waits, on_update=[])
                    else:
                        msi.on_wait = pending_waits + list(msi.on_wait)
                    pending_waits = None
        for i in reversed(to_delete):
            del blk.instructions[i]
        removed += len(to_delete)
    return removed


def _get_nc():
    if "nc" not in _CACHED:
        _CACHED["nc"] = _build_nc()
    return _CACHED["nc"]


def build_in_maps(inputs):
    x = np.asarray(inputs["x"], dtype=np.float32)
    ws = {l: np.ascontiguousarray(np.asarray(inputs[f"w{l}"], dtype=np.float32))
          for l in (1, 2, 3, 4)}
    bs = {l: np.ascontiguousarray(
        np.asarray(inputs[f"b{l}"], dtype=np.float32).reshape(-1, 128).T)
        for l in (1, 2, 3, 4)}
    in_maps = []
    for c in range(N_CORES):
        m = {
            "xst": np.ascontiguousarray(
                x[c * BS:(c + 1) * BS].T.astype(np.float16)),
            "w1s": ws[1][c * HS:(c + 1) * HS],
            "w2s": ws[2][c * HS:(c + 1) * HS],
            "w3s": ws[3][c * HS:(c + 1) * HS],
            "w4s": ws[4][c * OS:(c + 1) * OS],
            "b1": bs[1], "b2": bs[2], "b3": bs[3], "b4": bs[4],
        }
        in_maps.append(m)
    return in_maps


def kernel(**inputs):
    # stale collective/DMA state from a prior run on these cores can corrupt
    # AllGather results (observed: exec completes, output garbage); ask the
    # runtime for a core reset before it initializes
    os.environ.setdefault("NEURON_RT_RESET_CORES", "1")
    from concourse.bass_utils import run_bass_kernel_spmd

    nc = _get_nc()
    in_maps = build_in_maps(inputs)
    res = run_bass_kernel_spmd(nc, in_maps, list(range(N_CORES)))
    out = np.empty((B, OUT), dtype=np.float32)
    for c in range(N_CORES):
        out[c * BS:(c + 1) * BS] = res.results[c]["y"].T
    return out


if __name__ == "__main__":
    rng = np.random.default_rng(0)
    ins = {
        "x": rng.standard_normal((B, IN)).astype(np.float32),
        "w1": (rng.standard_normal((H, IN)) * 0.1).astype(np.float32),
        "b1": np.zeros(H, np.float32),
        "w2": (rng.standard_normal((H, H)) * 0.1).astype(np.float32),
        "b2": np.zeros(H, np.float32),
        "w3": (rng.standard_normal((H, H)) * 0.1).astype(np.float32),
        "b3": np.zeros(H, np.float32),
        "w4": (rng.standard_normal((OUT, H)) * 0.1).astype(np.float32),
        "b4": np.zeros(OUT, np.float32),
    }
    y = kernel(**ins)
    print("kernel ran, output shape", y.shape, "mean", float(y.mean()))



# revision 49
# speedup vs baseline: 1.0085x; 1.0085x over previous
"""FP4Net (bnb-FP4 quantize-dequantize 4-layer MLP) Trainium2 kernel.

Strategy (8 NeuronCores):
  - Data-parallel over batch for the matmuls: each core handles 1024 of 8192 rows.
  - FP4 quant-dequant of the weights is sharded 8x across cores (by output-row
    blocks, keeping the 64-elem FP4 blocks intact), computed exactly with fp32
    bit tricks on the vector engine, stored transposed (W.T layout) in fp16,
    then AllGathered so every core has all dequantized weights.
  - Each weight's AllGather is split into single-128-row-block chunks (w1/w2/
    w3: 4 each, w4: 1); chunk ci gathers j-tiles {c*nrt+r for cores c, r in
    chunk}, and each layer's j-loop walks chunks in order, so matmuls start as
    soon as the first chunk lands. Each AllGather is emitted on the gpsimd
    queue immediately after its chunk's stores (earliest possible arm; the
    mesh start is bounded by ~45us of cross-core launch skew, not by us).
    Short chunks keep each collective's HBM-saturating transfer window small
    so the PE's weight-strip DMAs recover between windows.
  - Queue discipline: Vector = dequant math only; Scalar(ACT) = dequant input
    loads + x/bias staging + epilogues; GpSimd = stores + AllGathers;
    SP(sync) = weight-strip loads + output stores (pure PE feed).
  - Dequant-side work is emitted into the layer loops by estimated ready time,
    so no queue head-of-line blocks on a dependency that isn't about to be
    satisfied.
  - Matmuls measure 262.6ns per [128,128]x[128,512] on this platform: a
    SW/GPIO HAM throttle caps the PE at 13/16 duty (1.95GHz) under sustained
    8-core load, and any PE idle >3.4us re-throttles to 4/8 with a slow
    recovery -- so the schedule optimizes for an unbroken matmul stream:
    ~270 garbage warm-up matmuls bridge the collective-bound startup (PE
    would otherwise idle ~60us), deep strip double-buffering (wt 5, wt1 10)
    rides out DMA starvation during AllGather windows, and redundant
    per-matmul LDWEIGHTS are deduped post-scheduling (_dedup_ldweights).

Rounding trick: with g = 3*w/scale, the bnb FP4 codebook {0, 1/192, 1/6, 1/4,
1/3, 1/2, 2/3, 1} maps to {0, 1/64, 1/2, 3/4, 1, 3/2, 2, 3}: round-to-nearest
over that set == round g to 1 stored mantissa bit (round-half-up via exact
small-significand integer adds), clamped below at 1/2, plus a two-threshold
step for the {0, 1/64} region. Verified bit-exact vs the jax reference modulo
~1-ulp boundary fuzz (~1 flipped element per 16M weights on the actual data).
"""
import os
import sys
import numpy as np

for _p in ("/opt/trn_rl_repo", "/root/.axon_site/_ro/trn_rl_repo"):
    if _p not in sys.path:
        sys.path.append(_p)

N_CORES = 8
B, IN, H, OUT = 8192, 1024, 4096, 1024
BS = B // N_CORES          # batch shard per core
HS = H // N_CORES          # hidden-row shard per core (w1/w2/w3)
OS = OUT // N_CORES        # out-row shard per core (w4)

# FP4 codebook-derived threshold constants (g-space = 3*norm), f64 precision
_FP4_POS = np.array([0.0, 0.0052083333, 0.6666667, 1.0, 0.3333333, 0.5,
                     0.1666667, 0.25], dtype=np.float32)
_CS = np.sort(_FP4_POS).astype(np.float64)
_TL = float(np.float32(3.0 * (_CS[0] + _CS[1]) / 2.0))
_TH = float(np.float32(3.0 * (_CS[1] + _CS[2]) / 2.0))
LO_BITS = int(np.float32(1.0 / 64).view(np.uint32))   # 0x3C800000
BIG_BITS = 0x40400000                                  # bits of 3.0


def _i32(x):
    return int(np.uint32(x).view(np.int32))


_CACHED = {}

# weight dims per layer: (rows of W == dout, k == contraction)
WDIMS = {1: (H, IN), 2: (H, H), 3: (H, H), 4: (OUT, H)}
NRT = {l: (d // N_CORES) // 128 for l, (d, _k) in WDIMS.items()}  # r-blocks
CHUNKS = {1: [[0], [1], [2], [3]], 2: [[0], [1], [2], [3]],
          3: [[0], [1], [2], [3]], 4: [[0]]}      # r-blocks per AllGather chunk
# k-segments per (layer, chunk): AllGather k-splitting (unused -- splitting
# w2c0 made its tail land LATER due to CC serialization + per-AG setup cost)
SEGS = {}
FDQ = 512          # dequant tile free-size (fp32 elems per partition)
NBQ = FDQ // 64    # fp4 blocks per tile

# --- static timing model (us) used only to order emission ---
DVE_TILE_US = 7.5          # dequant DVE time per [128, FDQ] tile
DVE_T0_US = 5.0            # engine init before first dequant op
MM_US = 0.2626             # per N=512 matmul at HAM 13/16 duty
L1_START_US = 46.0         # estimated first-epilogue time of layer 1
EV_MARGIN_US = 3.0


def _build_nc(taps=False):
    import concourse.bass as bass
    import concourse.mybir as mybir
    import concourse.tile as tile
    from concourse import bacc

    dt = mybir.dt
    Alu = mybir.AluOpType
    Act = mybir.ActivationFunctionType

    nc = bacc.Bacc("TRN2", target_bir_lowering=False, debug=False,
                   num_devices=N_CORES)

    # ---- I/O ----
    xs = nc.dram_tensor("xst", [IN, BS], dt.float16, kind="ExternalInput")
    w_in = {
        1: nc.dram_tensor("w1s", [HS, IN], dt.float32, kind="ExternalInput"),
        2: nc.dram_tensor("w2s", [HS, H], dt.float32, kind="ExternalInput"),
        3: nc.dram_tensor("w3s", [HS, H], dt.float32, kind="ExternalInput"),
        4: nc.dram_tensor("w4s", [OS, H], dt.float32, kind="ExternalInput"),
    }
    b_in = {
        1: nc.dram_tensor("b1", [128, H // 128], dt.float32, kind="ExternalInput"),
        2: nc.dram_tensor("b2", [128, H // 128], dt.float32, kind="ExternalInput"),
        3: nc.dram_tensor("b3", [128, H // 128], dt.float32, kind="ExternalInput"),
        4: nc.dram_tensor("b4", [128, OUT // 128], dt.float32, kind="ExternalInput"),
    }
    y_out = nc.dram_tensor("y", [OUT, BS], dt.float32, kind="ExternalOutput")


    # ---- internal DRAM: per-chunk-segment shard + AllGather outputs ----
    dq_shard = {}   # (l, ci, s) -> local tensor [len(chunk), kseg, 128]
    dq_chunk = {}   # (l, ci, s) -> gathered [N_CORES*len(chunk), kseg, 128]
    for l, (dout, k) in WDIMS.items():
        for ci, chunk in enumerate(CHUNKS[l]):
            ns = SEGS.get((l, ci), 1)
            for s in range(ns):
                dq_shard[(l, ci, s)] = nc.dram_tensor(
                    f"dqs{l}{ci}{s}", [len(chunk), k // ns, 128], dt.float16)
                dq_chunk[(l, ci, s)] = nc.dram_tensor(
                    f"dqc{l}{ci}{s}", [N_CORES * len(chunk), k // ns, 128],
                    dt.float16, addr_space="Shared")

    with tile.TileContext(nc) as tc:
        with (
            tc.tile_pool(name="const", bufs=1) as cpool,
            tc.tile_pool(name="bias", bufs=1) as bpool,
            tc.tile_pool(name="a0", bufs=1) as a0pool,
            tc.tile_pool(name="acts", bufs=2) as apool,
            tc.tile_pool(name="dqin", bufs=3) as dqin_pool,
            tc.tile_pool(name="dqtmp", bufs=1) as dqtmp_pool,
            tc.tile_pool(name="dqout", bufs=3) as dqout_pool,
            tc.tile_pool(name="stp", bufs=3) as stp_pool,
            tc.tile_pool(name="wt", bufs=5) as wpool,
            tc.tile_pool(name="wt1", bufs=5) as wpool1,
            tc.tile_pool(name="psum", bufs=8, space="PSUM") as pspool,
        ):
            # int32 constants for scalar_tensor_tensor scalars
            c_half = cpool.tile([128, 1], dt.int32)
            nc.vector.memset(c_half[:], _i32(0x3F000000))
            c_sign = cpool.tile([128, 1], dt.int32)
            nc.vector.memset(c_sign[:], _i32(0x80000000))
            wz = cpool.tile([128, 128], dt.float16)
            nc.vector.memset(wz[:], 0.0)

            # ---- dequant pipeline pieces ----
            # tile order: (l, r, cix) over all weights, r-block-major
            all_tiles = []
            for l in WDIMS:
                for r in range(NRT[l]):
                    for cix in range(WDIMS[l][1] // FDQ):
                        all_tiles.append((l, r, cix))
            t_index = {t: i for i, t in enumerate(all_tiles)}

            def dve_done(i):  # estimated DVE completion time of tile i (us)
                return DVE_T0_US + (i + 1) * DVE_TILE_US

            def emit_dq_load(l, r, cix):
                """Scalar queue: stage [128 rows, FDQ] fp32 of weight l."""
                w = dqin_pool.tile([128, NBQ, 64], dt.float32, tag="dqw")
                nc.scalar.dma_start(
                    w[:],
                    w_in[l][r * 128:(r + 1) * 128, cix * FDQ:(cix + 1) * FDQ]
                    .rearrange("p (b i) -> p b i", i=64))
                return w

            def emit_dq_vec_pair(tiles, staged):
                """Vector queue: dequant math + 32x32 stream-transpose for a
                pair of tiles, op-interleaved so the per-op semaphore latency
                of one tile's chain hides behind the other tile's op."""
                ss = []
                for par, t in enumerate(tiles):
                    w = staged.pop(t)

                    def tmp(nm, shape, dty, par=par):
                        return dqtmp_pool.tile(shape, dty, name=f"{nm}{par}",
                                               tag=f"{nm}{par}")
                    ss.append({"w": w, "tmp": tmp})

                def each(fn):
                    for s in ss:
                        fn(s)

                def st_scale(s):
                    s["scale"] = s["tmp"]("scale", [128, NBQ, 1], dt.float32)
                    nc.vector.tensor_reduce(
                        s["scale"][:], s["w"][:], axis=mybir.AxisListType.X,
                        op=Alu.max, apply_absolute_value=True)
                each(st_scale)

                def st_recip(s):
                    s["recip"] = s["tmp"]("recip", [128, NBQ, 1], dt.float32)
                    nc.vector.reciprocal(s["recip"][:], s["scale"][:])
                each(st_recip)

                def st_s3(s):
                    s["s3"] = s["tmp"]("s3", [128, NBQ, 1], dt.float32)
                    nc.vector.tensor_scalar_mul(s["s3"][:], s["scale"][:],
                                                1.0 / 3.0)
                each(st_s3)

                def st_g(s):
                    s["g"] = s["tmp"]("g", [128, NBQ, 64], dt.float32)
                    nc.vector.scalar_tensor_tensor(
                        s["g"][:], s["w"][:], 3.0,
                        s["recip"][:].broadcast_to((128, NBQ, 64)),
                        op0=Alu.mult, op1=Alu.mult)
                each(st_g)

                # NOTE: DVE ops must never write in-place onto their own input
                # (dual-port perf modes race), and int adds must keep few
                # significant bits (the int ALU path is fp32-internal).
                def st_ta(s):
                    s["ta"] = s["tmp"]("ta", [128, NBQ, 64], dt.int32)
                    nc.vector.tensor_scalar(
                        s["ta"][:], s["g"][:].bitcast(dt.int32),
                        _i32(0x7FFFFFFF), None,
                        op0=Alu.bitwise_and)  # m0 = |g| bits
                each(st_ta)

                def st_tb(s):
                    s["tb"] = s["tmp"]("tb", [128, NBQ, 64], dt.int32)
                    nc.vector.tensor_scalar(s["tb"][:], s["ta"][:],
                                            _i32(0xFFC00000), None,
                                            op0=Alu.bitwise_and)  # trunc
                each(st_tb)

                def st_tc(s):
                    s["tc"] = s["tmp"]("tc", [128, NBQ, 64], dt.int32)
                    nc.vector.tensor_scalar(
                        s["tc"][:], s["ta"][:], _i32(0x00200000), _i32(1),
                        op0=Alu.bitwise_and,
                        op1=Alu.logical_shift_left)  # half-bit<<1
                each(st_tc)

                def st_te(s):
                    s["te"] = s["tmp"]("te", [128, NBQ, 64], dt.int32)
                    nc.vector.tensor_tensor(
                        s["te"][:], s["tb"][:], s["tc"][:],
                        op=Alu.add)  # r2a (exact: 10+1 sig bits)
                each(st_te)

                # M1L = (|g|>TL)*LO_BITS, M2B = (|g|>TH)*BIG_BITS -- the float
                # products are exact (consts have <=5 significant bits)
                def st_m1(s):
                    af = s["ta"][:].bitcast(dt.float32)  # |g| as float
                    nc.vector.tensor_scalar(s["tb"][:], af, _TL,
                                            float(LO_BITS),
                                            op0=Alu.is_gt, op1=Alu.mult)
                each(st_m1)

                def st_m2(s):
                    # ta (|g| bits) has no reader after this: m2 -> tc and the
                    # select result can overwrite ta, freeing the former td
                    # buffer entirely (SBUF pressure).
                    af = s["ta"][:].bitcast(dt.float32)
                    nc.vector.tensor_scalar(s["tc"][:], af, _TH,
                                            float(BIG_BITS),
                                            op0=Alu.is_gt, op1=Alu.mult)
                each(st_m2)

                def st_sel(s):
                    nc.vector.tensor_tensor(s["ta"][:], s["tb"][:], s["tc"][:],
                                            op=Alu.add)  # sel (disjoint bits)
                each(st_sel)

                def st_mag(s):
                    nc.vector.scalar_tensor_tensor(
                        s["tb"][:], s["te"][:], c_half[:], s["ta"][:],
                        op0=Alu.max, op1=Alu.min)  # mag
                each(st_mag)

                def st_sign(s):
                    nc.vector.scalar_tensor_tensor(
                        s["tc"][:], s["g"][:].bitcast(dt.int32), c_sign[:],
                        s["tb"][:],
                        op0=Alu.bitwise_and, op1=Alu.bitwise_or)  # signed
                each(st_sign)

                def st_dq(s):
                    s["dq"] = dqout_pool.tile([128, NBQ, 64], dt.float16,
                                              name="dq", tag="dq")
                    nc.vector.tensor_tensor(
                        s["dq"][:], s["tc"][:].bitcast(dt.float32),
                        s["s3"][:].broadcast_to((128, NBQ, 64)), op=Alu.mult)
                each(st_dq)

                # 32x32 block transpose on the DVE reshape front-end:
                # st[32a+u, 32b+v] = dq[32a+v, 32b+u]
                def st_tp(s):
                    s["st"] = stp_pool.tile([128, FDQ], dt.float16,
                                            name="st", tag="st")
                    nc.vector.transpose(
                        s["st"][:], s["dq"][:].rearrange("p b i -> p (b i)"))
                each(st_tp)
                return [s["st"] for s in ss]

            ntk = {l: WDIMS[l][1] // FDQ for l in WDIMS}

            def chunk_of(l, r):
                return next(c for c, ch in enumerate(CHUNKS[l]) if r in ch)

            def seg_of(l, ci, cix):
                ns = SEGS.get((l, ci), 1)
                return cix // (ntk[l] // ns)

            def emit_dq_store(l, ci, r, cix, st, split=False):
                """GpSimd: 4 block-grid-swapped stores complete the transpose.
                dst[k=cix*FDQ+32b+u, h=32a+v] = st[32a+u, 32b+v]. For the
                first tiles (split=True) half the stores ride the still-idle
                sync queue, so the first AllGather's data is ready sooner."""
                ri = r - CHUNKS[l][ci][0]
                s = seg_of(l, ci, cix)
                cis = cix - s * (ntk[l] // SEGS.get((l, ci), 1))
                for a in range(4):
                    q = nc.sync if (split and a % 2) else nc.gpsimd
                    q.dma_start(
                        dq_shard[(l, ci, s)][ri, cis * FDQ:(cis + 1) * FDQ,
                                             32 * a:32 * (a + 1)]
                        .rearrange("(f p) h -> p f h", p=32),
                        st[32 * a:32 * (a + 1), :]
                        .rearrange("p (f i) -> p f i", i=32))

            def emit_ag(l, ci, s):
                nc.gpsimd.collective_compute(
                    "AllGather", Alu.bypass,
                    replica_groups=[list(range(N_CORES))],
                    ins=[dq_shard[(l, ci, s)][:]],
                    outs=[dq_chunk[(l, ci, s)][:]],
                )

            # ---- event list: (ready_us, seq, kind, payload), ready-ordered.
            # AllGathers are NOT events: each is emitted inline right after
            # the stores of its chunk-segment's last tile, so it sits in the
            # gpsimd queue (and thus the CC engine) at the earliest moment
            # its data can be complete -- no head-of-line behind later
            # tiles' stores.
            events = []
            seq = 0
            N_HOIST = 3   # dq loads emitted before x/bias staging
            for t in all_tiles:
                i = t_index[t]
                rdy_load = 0.0 if i < 4 else dve_done(i - 4)
                if i >= N_HOIST:
                    events.append((rdy_load, seq, "load", t)); seq += 1
                if i % 2 == 1:
                    events.append((rdy_load, seq, "vec",
                                   (all_tiles[i - 1], t))); seq += 1
            events.sort(key=lambda e: (e[0], e[1]))
            events = events[::-1]  # pop from end

            staged = {}   # tile -> dqin handle

            def flush_events(now_us):
                while events and events[-1][0] <= now_us:
                    _, _, kind, payload = events.pop()
                    if kind == "load":
                        staged[payload] = emit_dq_load(*payload)
                    else:
                        sts = emit_dq_vec_pair(payload, staged)
                        for t, st in zip(payload, sts):
                            l, r, cix = t
                            ci = chunk_of(l, r)
                            emit_dq_store(l, ci, r, cix, st,
                                          split=t_index[t] < 4)
                            ns = SEGS.get((l, ci), 1)
                            s = seg_of(l, ci, cix)
                            if (r == CHUNKS[l][ci][-1]
                                    and cix == (s + 1) * (ntk[l] // ns) - 1):
                                emit_ag(l, ci, s)

            # loads for the first tiles go out before x/bias staging so the
            # dequant chain (the critical path to the first AllGather) starts
            # as early as possible
            for t in all_tiles[:N_HOIST]:
                staged[t] = emit_dq_load(*t)

            # ---- biases -> SBUF [128, ntiles]; x -> SBUF a0 (Scalar queue) ----
            b_sb = {}
            for l, (dout, _k) in WDIMS.items():
                nj = dout // 128
                bt = bpool.tile([128, nj], dt.float32, tag=f"bias{l}")
                nc.scalar.dma_start(bt[:], b_in[l][:])
                b_sb[l] = bt
            a_cur = a0pool.tile([128, IN // 128, BS], dt.float16)
            nc.scalar.dma_start(
                a_cur[:], xs[:].rearrange("(i p) b -> p i b", p=128))

            # everything ready before the first epilogue goes out up front
            flush_events(L1_START_US)

            # ---- HAM warm-up: garbage matmuls through the collective-bound
            # startup window (PE would idle ~80us waiting for the first
            # AllGather; idle >3.4us re-throttles the PE clock to 4/8, taxing
            # the first real j-blocks). Zero weights, x as moving operand,
            # results discarded. The ldweights dedup collapses these to one
            # weight load.
            for _wu in range(270):
                ps_w = pspool.tile([128, 512], dt.float32, tag="ps")
                nc.tensor.matmul(ps_w[:], wz[:], a_cur[:, 0, 0:512],
                                 start=True, stop=True)

            # ---- matmul layers, chunk-ordered j loops ----
            t_us = L1_START_US
            for l, (dout, K) in WDIMS.items():
                nj = dout // 128
                nk = K // 128
                nrt = NRT[l]
                half = nk // 2
                out_dt = dt.float32 if l == 4 else dt.float16
                a_next = apool.tile([128, nj, BS], out_dt, tag="acts")
                j_period = nk * (BS // 512) * MM_US
                for ci, chunk in enumerate(CHUNKS[l]):
                    for c in range(N_CORES):
                        for ri, r in enumerate(chunk):
                            j = c * nrt + r
                            row = c * len(chunk) + ri
                            src = dq_chunk[(l, ci, 0)][row]
                            if l == 1:
                                # one small DMA per j-block: a single
                                # descriptor through the arbiter, so the
                                # whole j-block's weights arrive together
                                # even inside an AllGather transfer window
                                wt_h = wpool1.tile([128, nk, 128],
                                                   dt.float16, tag="wt1")
                                nc.sync.dma_start(
                                    wt_h[:],
                                    src[:].rearrange("(i p) h -> p i h",
                                                     p=128))
                                wsel = (lambda i, w=wt_h: w[:, i, :])
                            else:
                                wts = []
                                for si, i0 in enumerate((0, half)):
                                    wt_h = wpool.tile([128, half, 128],
                                                      dt.float16, tag="wt")
                                    # second strip rides the (mostly idle)
                                    # scalar queue: two independent queue
                                    # positions in the DMA arbiter halve
                                    # strip starvation during AllGather
                                    # HBM-saturation windows
                                    q = nc.sync if si == 0 else nc.scalar
                                    q.dma_start(
                                        wt_h[:],
                                        src[i0 * 128:(i0 + half) * 128, :]
                                        .rearrange("(i p) h -> p i h", p=128))
                                    wts.append(wt_h)
                                wsel = (lambda i, w=wts:
                                        w[i // half][:, i % half, :])
                            ps = []
                            for _n in range(BS // 512):
                                ps_t = pspool.tile([128, 512], dt.float32,
                                                   tag="ps")
                                ps.append(ps_t)
                            for i in range(nk):
                                for n in range(BS // 512):
                                    nc.tensor.matmul(
                                        ps[n][:], wsel(i),
                                        a_cur[:, i, n * 512:(n + 1) * 512],
                                        start=(i == 0), stop=(i == nk - 1))
                            t_us += j_period
                            flush_events(t_us)
                            act_fn = Act.Sigmoid if l == 4 else Act.Relu
                            for n in range(BS // 512):
                                nc.scalar.activation(
                                    a_next[:, j, n * 512:(n + 1) * 512],
                                    ps[n][:], act_fn,
                                    bias=b_sb[l][:, j:j + 1], scale=1.0)
                                if l == 4:
                                    nc.sync.dma_start(
                                        y_out[j * 128:(j + 1) * 128,
                                              n * 512:(n + 1) * 512],
                                        a_next[:, j, n * 512:(n + 1) * 512])
                a_cur = a_next
            flush_events(1e9)

    _dedup_ldweights(nc, mybir)
    nc.compile()
    return nc


def _dedup_ldweights(nc, mybir):
    """Remove back-to-back duplicate InstLdweights (same weights AP).

    The tile scheduler pairs every InstMatmult with its own InstLdweights,
    but the PE's background weight buffer only frees when the next matmul
    issues (buffer flip), serializing flip->ldweights(110ns)->handshake
    (62ns) = 263ns per matmul. Our two batch-half matmuls share a weight
    tile: dropping the second (redundant) load makes the second matmul
    flip-free and hides the next load under 2x213ns of array work, so the
    stream becomes array-bound. Waits on a dropped load migrate to the
    following matmul; program-order weight semantics are preserved.
    """
    fn = nc.m.functions[0]
    removed = 0
    for blk in fn.blocks:
        insts = list(blk.instructions)
        last_sig = None
        pending_waits = None
        to_delete = []
        for i, inst in enumerate(insts):
            nm = type(inst).__name__
            if nm == "InstLdweights":
                sig = str(inst.ins[0])
                if sig == last_sig:
                    si = inst.sync_info
                    if si is not None and len(si.on_update) > 0:
                        last_sig = sig  # has updates: keep it, don't merge
                        continue
                    if si is not None and len(si.on_wait) > 0:
                        pending_waits = list(si.on_wait)
                    to_delete.append(i)
                else:
                    last_sig = sig
            elif nm == "InstMatmult":
                if pending_waits:
                    msi = inst.sync_info
                    if msi is None:
                        inst.sync_info = mybir.SyncInfo(
                            on_wait=pending_waits, on_update=[])
                    else:
                        msi.on_wait = pending_waits + list(msi.on_wait)
                    pending_waits = None
        for i in reversed(to_delete):
            del blk.instructions[i]
        removed += len(to_delete)
    return removed


def _get_nc():
    if "nc" not in _CACHED:
        _CACHED["nc"] = _build_nc()
    return _CACHED["nc"]


def build_in_maps(inputs):
    x = np.asarray(inputs["x"], dtype=np.float32)
    ws = {l: np.ascontiguousarray(np.asarray(inputs[f"w{l}"], dtype=np.float32))
          for l in (1, 2, 3, 4)}
    bs = {l: np.ascontiguousarray(
        np.asarray(inputs[f"b{l}"], dtype=np.float32).reshape(-1, 128).T)
        for l in (1, 2, 3, 4)}
    in_maps = []
    for c in range(N_CORES):
        m = {
            "xst": np.ascontiguousarray(
                x[c * BS:(c + 1) * BS].T.astype(np.float16)),
            "w1s": ws[1][c * HS:(c + 1) * HS],
            "w2s": ws[2][c * HS:(c + 1) * HS],
            "w3s": ws[3][c * HS:(c + 1) * HS],
            "w4s": ws[4][c * OS:(c + 1) * OS],
            "b1": bs[1], "b2": bs[2], "b3": bs[3], "b4": bs[4],
        }
        in_maps.append(m)
    return in_maps


def kernel(**inputs):
    # stale collective/DMA state from a prior run on these cores can corrupt
    # AllGather results (observed: exec completes, output garbage); ask the
    # runtime for a core reset before it initializes
    os.environ.setdefault("NEURON_RT_RESET_CORES", "1")
    from concourse.bass_utils import run_bass_kernel_spmd

    nc = _get_nc()
    in_maps = build_in_maps(inputs)
    res = run_bass_kernel_spmd(nc, in_maps, list(range(N_CORES)))
    out = np.empty((B, OUT), dtype=np.float32)
    for c in range(N_CORES):
        out[c * BS:(c + 1) * BS] = res.results[c]["y"].T
    return out


if __name__ == "__main__":
    rng = np.random.default_rng(0)
    ins = {
        "x": rng.standard_normal((B, IN)).astype(np.float32),
        "w1": (rng.standard_normal((H, IN)) * 0.1).astype(np.float32),
        "b1": np.zeros(H, np.float32),
        "w2": (rng.standard_normal((H, H)) * 0.1).astype(np.float32),
        "b2": np.zeros(H, np.float32),
        "w3": (rng.standard_normal((H, H)) * 0.1).astype(np.float32),
        "b3": np.zeros(H, np.float32),
        "w4": (rng.standard_normal((OUT, H)) * 0.1).astype(np.float32),
        "b4": np.zeros(OUT, np.float32),
    }
    y = kernel(**ins)
    print("kernel ran, output shape", y.shape, "mean", float(y.mean()))

